# revision 18
# baseline (speedup 1.0000x reference)
"""Trainium2 Bass kernel for the 20-layer dilated-causal-conv audio model.

Formulation (validated against the reference in numpy):
- Only the last 128 output timesteps are needed -> per-layer suffix pyramid.
  Layer i only computes timesteps in blocks [TB[i+1], 512) of 16 steps each.
- Channels are tiny (8), so convs run on the TensorEngine as block-Toeplitz
  matmuls: partition dim = 16 timesteps x 8 channels = 128; each conv tap is a
  host-built 128x128 stationary matrix; taps accumulate in PSUM.
- The per-layer control (1x1 conv on ctrl) is one extra matmul with
  contraction 16; per-channel constants (conv bias, ctrl bias, folded io_b
  drift) ride the ReLU (activation bias on Act, tensor_scalar on Pool).
- Residual 1x1 (io_w) is a block-diagonal matmul; the residual add runs on
  DVE/Pool. The final mixer is a per-layer [128,16] matmul over the last 8
  blocks accumulated in one PSUM group per chain.
- bf16 storage/matmuls (PSUM accumulate fp32): 1 cycle/row on PE with no
  small-matmul penalty, half the DMA bytes, 2x DVE throughput.
- All 128-partition weights live in ONE layer-major packed DRAM tensor
  (taps_i | iow_i | mixw_i per layer) moved by two large DMAs; 16-partition
  stationaries (ctrl broadcast, layer-0 taps, audio inject) in one aux
  tensor. Few DMAs = little engine issue time + low descriptor overhead.
- Data parallel over batch: 32 batches -> 8 cores x 4; per core 4 chains of
  1 batch element pipeline across engines to hide the 20-layer serial path.
"""

import numpy as np
import ml_dtypes

import concourse.bass as bass
import concourse.mybir as mybir
import concourse.tile as tile
from concourse.bass_utils import run_bass_kernel_spmd

# ---------------------------------------------------------------- constants
DIL = [1, 2, 4, 8, 16, 32, 64, 128, 256, 512] * 2
NL = 20          # layers
CH = 8           # channels
BLK = 16         # timesteps per block
NB = 512         # blocks in T=8192
T = 8192
B = 32           # total batch
NCORES = 8
BPC = B // NCORES  # batch per core
NCHAINS = 2      # independent batch chains (latency hiding); bpc_c = BPC//NCHAINS

DT = mybir.dt.bfloat16
NPDT = ml_dtypes.bfloat16

# engine per chain for relu (PSUM->SBUF) and residual add.
# NOTE: GPSIMD/Pool cannot access PSUM on TRN2 (BIR verifier rejects it),
# so both stages are limited to Act (activation) and DVE (tensor ops).
RELU_ENG = ["act", "dve", "act", "dve"]
ADD_ENG = ["dve", "dve", "dve", "dve"]

# block-start table: TB[i] = first block of x~_i ; TB[NL] = first output block.
# Extents (NB - TB[i]) forced even (even innermost free counts keep matmul
# APs 4-byte aligned in bf16).
TB = [0] * (NL + 1)
TB[NL] = NB - 8
for i in range(NL - 1, -1, -1):
    TB[i] = TB[i + 1] - max(1, (2 * DIL[i]) // BLK)
    if (NB - TB[i]) % 2:
        TB[i] -= 1

# per-layer tap metadata: list of (block_offset, tap_index_within_layer)
_TAP_OFFSETS = []
for _i in range(NL):
    d = DIL[_i]
    offs = [0, d // BLK, 2 * (d // BLK)] if d >= BLK else [0, 1]
    _TAP_OFFSETS.append(offs)

# ---- packed column layout
# wall[128, NW]: per-layer blocks in use order:
#   layer 0: iow_0 (128) | mixw_0 (16)
#   layer i>0: taps_i (ntaps*128) | iow_i (128, absent for i=19) | mixw_i (16)
# aux[17, NAUX]: slot layout (128 cols each):
#   slots 0..NL-1: ctrl broadcast (scaled by ctrl_w[i]) for layer i; row 16
#                  carries the per-layer bias vector (the ctrl moving tile has
#                  a host-built ones row 16, so the bias lands in PSUM free)
#   slot NL:      audio->8ch broadcast (layer-0 residual inject)
#   slots NL+1, NL+2: layer-0 taps (cin=1)
_WCOL_TAP = {}   # (i, j) -> col of 128-wide tap matrix for layer i>0
_WCOL_IOW = {}   # i -> col
_WCOL_MIX = {}   # i -> col
_nw = 0
for _i in range(NL):
    if _i > 0:
        for _j in range(len(_TAP_OFFSETS[_i])):
            _WCOL_TAP[(_i, _j)] = _nw
            _nw += 128
    if _i < NL - 1:
        _WCOL_IOW[_i] = _nw
        _nw += 128
    _WCOL_MIX[_i] = _nw
    _nw += 16
NW = _nw
# wall DMA split points: [0, layer-3, layer-9) segments on SP/Act/Pool queues
_WSPLIT1 = _WCOL_TAP[(3, 0)]
_WSPLIT2 = _WCOL_TAP[(9, 0)]

NAUX = (NL + 3) * 128
_ACOL_CTRL = [i * 128 for i in range(NL)]
_ACOL_AUD = NL * 128
_ACOL_TAP0 = [(NL + 1) * 128, (NL + 2) * 128]


# ------------------------------------------------- workaround: 1-wait limit
def _split_multi_waits(nc):
    """This walrus build allows only one sem wait per TPB instruction, but
    Tile's kernel-tail drain carries several. Move extras onto preceding
    same-engine nops (in-order execution keeps the gating semantics)."""
    tpb = {
        mybir.EngineType.SP,
        mybir.EngineType.PE,
        mybir.EngineType.DVE,
        mybir.EngineType.Activation,
        mybir.EngineType.Pool,
    }
    for f in nc.m.functions:
        for bb in f.blocks:
            new_list = []
            changed = False
            for inst in bb.instructions:
                si = inst.sync_info
                if si is not None and si.on_wait and len(si.on_wait) > 1 and inst.engine in tpb:
                    waits = list(si.on_wait)
                    for j, w in enumerate(waits[:-1]):
                        nop = mybir.InstNoOp(name=f"{inst.name}-ws{j}", ins=[], outs=[])
                        nop.engine = inst.engine
                        nop.sync_info = mybir.SyncInfo(on_wait=[w], on_update=[])
                        new_list.append(nop)
                    si.on_wait = waits[-1:]
                    changed = True
                new_list.append(inst)
            if changed:
                bb.instructions[:] = new_list


# ------------------------------------------------------------- host arrays
def _build_host_arrays(inputs):
    c_w0 = np.asarray(inputs["c_w0"], np.float32)    # [3,1,8]
    c_ws = np.asarray(inputs["c_ws"], np.float32)    # [19,3,8,8]
    c_b = np.asarray(inputs["c_b"], np.float32)      # [20,8]
    ctrl_w = np.asarray(inputs["ctrl_w"], np.float32)  # [20,1,1]
    ctrl_b = np.asarray(inputs["ctrl_b"], np.float32)  # [20,1]
    io_w = np.asarray(inputs["io_w"], np.float32)    # [19,8,8]
    io_b = np.asarray(inputs["io_b"], np.float32)    # [19,8]
    mix_w = np.asarray(inputs["mix_w"], np.float32)  # [160,1]

    wall = np.zeros((128, NW), np.float32)
    aux = np.zeros((17, NAUX), np.float32)

    for t in range(BLK):
        aux[t, _ACOL_AUD + t * 8 : _ACOL_AUD + t * 8 + 8] = 1.0

    const_i = np.zeros(CH, np.float32)
    for i in range(NL):
        w = c_w0 if i == 0 else c_ws[i - 1]          # [3, cin, 8]
        cin = w.shape[1]
        d = DIL[i]
        wD = [w[2], w[1], w[0]]                      # wD[l] multiplies x[t - l*d]
        bias = c_b[i] + ctrl_b[i][0]
        if cin == CH:
            bias = bias + np.einsum("kco,c->o", w, const_i)
        aux[16, _ACOL_CTRL[i] : _ACOL_CTRL[i] + 128] = np.tile(bias, BLK)

        # tap matrices: layer 0 (cin=1) is [16,128] in aux; layers >0 are
        # [128,128] in wall. row index = ti (cin=1) or ti*8+ci.
        if i == 0:
            mats = [aux[:, c : c + 128] for c in _ACOL_TAP0]
        else:
            mats = [
                wall[:, _WCOL_TAP[(i, j)] : _WCOL_TAP[(i, j)] + 128]
                for j in range(len(_TAP_OFFSETS[i]))
            ]

        def rows(ti):
            return slice(ti, ti + 1) if cin == 1 else slice(ti * 8, ti * 8 + cin)

        if d >= BLK:
            for l in range(3):
                W = mats[l]
                for t in range(BLK):
                    W[rows(t), t * 8 : t * 8 + 8] = wD[l][:cin]
        else:
            Wc, Wp = mats[0], mats[1]
            for to in range(BLK):
                for l in range(3):
                    ti = to - l * d
                    if ti >= 0:
                        Wc[rows(ti), to * 8 : to * 8 + 8] += wD[l][:cin]
                    else:
                        Wp[rows(ti + BLK), to * 8 : to * 8 + 8] += wD[l][:cin]

        for t in range(BLK):
            aux[t, _ACOL_CTRL[i] + t * 8 : _ACOL_CTRL[i] + t * 8 + 8] = ctrl_w[i][0, 0]
            wall[t * 8 : t * 8 + 8, _WCOL_MIX[i] + t] = mix_w[i * 8 : i * 8 + 8, 0]
        if i < NL - 1:
            for t in range(BLK):
                wall[t * 8 : t * 8 + 8, _WCOL_IOW[i] + t * 8 : _WCOL_IOW[i] + t * 8 + 8] = io_w[i]
            const_i = const_i + io_b[i]

    return dict(
        wall=wall.astype(NPDT),
        aux=aux.astype(NPDT),
    )


# ----------------------------------------------------------- device program
_NC_CACHE = {}


def _build_nc(loop_k=None, ablate=()):
    """loop_k: dev-only probe mode — wrap the whole body in For_i(0, loop_k)
    so marginal per-iteration wall time on HW isolates kernel exec from the
    ~100ms dispatch floor. ablate: dev-only cost attribution — shrink a
    stage's work to near-zero while keeping the dependency graph:
    "dma" (2-col transfers), "mm" (2-col matmuls), "act", "dve"."""
    nc = bass.Bass()
    abl = set(ablate)
    f32 = mybir.dt.float32

    nblk0 = NB - TB[0]
    nblk1 = NB - TB[1]
    # audio/ctrl arrive host-blocked as [16=t-in-block, BPC, nblk]
    audio_h = nc.dram_tensor("audio", [BLK, BPC, nblk0], DT, kind="ExternalInput")
    ctrl_h = nc.dram_tensor("ctrl", [17, BPC, nblk1], DT, kind="ExternalInput")
    wall_h = nc.dram_tensor("wall", [128, NW], DT, kind="ExternalInput")
    aux_h = nc.dram_tensor("aux", [17, NAUX], DT, kind="ExternalInput")
    out_h = nc.dram_tensor("out", [BPC, 128], f32, kind="ExternalOutput")

    import contextlib

    inline_k = 1
    if isinstance(loop_k, tuple):  # (outer For_i count, inline copies per pass)
        loop_k, inline_k = loop_k
    elif loop_k and loop_k < 0:    # negative: inline replication (no back-edge)
        inline_k, loop_k = -loop_k, None

    with tile.TileContext(nc) as tc:
        with (
            tc.For_i(0, loop_k, 1) if loop_k else contextlib.nullcontext(),
            tc.tile_pool(name="w", bufs=2) as wpool,
            tc.tile_pool(name="xs", bufs=1) as xpool,
            tc.tile_pool(name="h", bufs=1) as hpool,
            tc.tile_pool(name="pc", bufs=4, space="PSUM") as pcpool,
            tc.tile_pool(name="pio", bufs=3, space="PSUM") as piopool,
            tc.tile_pool(name="pm", bufs=1, space="PSUM") as pmpool,
        ):
            for rep in range(inline_k):
                audio_t = xpool.tile([16, BPC, nblk0], DT, tag="x0", name="audio_t")
                ctrl_t = wpool.tile([17, BPC, nblk1], DT, name="ctrl_t")
                wall_t = wpool.tile([128, NW], DT, name="wall_t")
                aux_t = wpool.tile([17, NAUX], DT, name="aux_t")

                # all bulk DMAs ride SP (no compute there, so next
                # iteration's prefetch is never stuck behind engine work);
                # ctrl rides Act. gpsimd/SWDGE DMAs are avoided: walrus
                # codegen rejects them inside For_i loops (ISA wrong length).
                if "dma" in abl:
                    nc.sync.dma_start(out=aux_t[:, :2], in_=aux_h[:, :2])
                    nc.sync.dma_start(out=audio_t[:, :, :2], in_=audio_h[:, :, :2])
                    nc.scalar.dma_start(out=ctrl_t[:, :, :2], in_=ctrl_h[:, :, :2])
                    nc.sync.dma_start(out=wall_t[:, :2], in_=wall_h[:, :2])
                else:
                    nc.sync.dma_start(out=aux_t[:], in_=aux_h[:])
                    nc.sync.dma_start(out=audio_t[:], in_=audio_h[:])
                    nc.scalar.dma_start(out=ctrl_t[:], in_=ctrl_h[:])
                    nc.sync.dma_start(
                        out=wall_t[:, :_WSPLIT1], in_=wall_h[:, :_WSPLIT1]
                    )
                    nc.sync.dma_start(
                        out=wall_t[:, _WSPLIT1:_WSPLIT2],
                        in_=wall_h[:, _WSPLIT1:_WSPLIT2],
                    )
                    nc.sync.dma_start(
                        out=wall_t[:, _WSPLIT2:], in_=wall_h[:, _WSPLIT2:]
                    )


                # x_ts[c]: current x~ tile of chain c (the shared 16-partition
                # audio tile at layer 0); hs[c]: per-layer h tiles (persist for
                # the end-of-chain mixer pass)
                bpc_c = BPC // NCHAINS
                chunkb = 512 // bpc_c   # PSUM bank limit: bpc_c*w <= 512 f32
                x_ts = [audio_t] * NCHAINS
                hs = [[] for _ in range(NCHAINS)]
                for i in range(NL):
                    out_b = TB[i + 1]
                    ext = NB - out_b
                    for c in range(NCHAINS):
                        b0 = c * bpc_c
                        x_t = x_ts[c]
                        x_next = None
                        if i < NL - 1:
                            x_next = xpool.tile(
                                [128, bpc_c, ext], DT,
                                tag=f"x{c}_{i + 1}", name=f"x{c}_{i + 1}",
                            )
                        h = hpool.tile(
                            [128, bpc_c, ext], DT, tag=f"h{c}_{i}", name=f"h{c}_{i}"
                        )
                        hs[c].append((h, NB - 8 - out_b))
                        hi = NB
                        chunks = []
                        while hi > out_b:
                            clo = max(out_b, hi - chunkb)
                            chunks.append((clo, hi - clo))
                            hi = clo
                        for lo, w in chunks[::-1]:
                            wm = 2 if "mm" in abl else w
                            wa = 2 if "act" in abl else w
                            wd = 2 if "dve" in abl else w
                            o = lo - out_b
                            pc = pcpool.tile([128, bpc_c, chunkb], f32, name="pc")
                            for j, off in enumerate(_TAP_OFFSETS[i]):
                                a = lo - off - TB[i]
                                if i == 0:
                                    stat = aux_t[0:16, _ACOL_TAP0[j] : _ACOL_TAP0[j] + 128]
                                    mov = x_t[:, b0 : b0 + bpc_c, a : a + wm]
                                else:
                                    col = _WCOL_TAP[(i, j)]
                                    stat = wall_t[:, col : col + 128]
                                    mov = x_t[:, :, a : a + wm]
                                nc.tensor.matmul(
                                    pc[:, :, :wm], stat, mov, start=(j == 0), stop=False
                                )
                            a = lo - TB[1]
                            nc.tensor.matmul(
                                pc[:, :, :wm],
                                aux_t[0:17, _ACOL_CTRL[i] : _ACOL_CTRL[i] + 128],
                                ctrl_t[0:17, b0 : b0 + bpc_c, a : a + wm],
                                start=False,
                                stop=True,
                            )
                            if RELU_ENG[c] == "act":
                                nc.scalar.activation(
                                    out=h[:, :, o : o + wa],
                                    in_=pc[:, :, :wa],
                                    func=mybir.ActivationFunctionType.Relu,
                                )
                            else:
                                nc.vector.tensor_scalar(
                                    out=h[:, :, o : o + wa],
                                    in0=pc[:, :, :wa],
                                    scalar1=0.0,
                                    scalar2=None,
                                    op0=mybir.AluOpType.max,
                                )
                            if i < NL - 1:
                                pio = piopool.tile([128, bpc_c, chunkb], f32, name="pio")
                                col = _WCOL_IOW[i]
                                if i == 0:
                                    nc.tensor.matmul(
                                        pio[:, :, :wm],
                                        wall_t[:, col : col + 128],
                                        h[:, :, o : o + wm],
                                        start=True,
                                        stop=False,
                                    )
                                    a = lo - TB[0]
                                    nc.tensor.matmul(
                                        pio[:, :, :wm],
                                        aux_t[0:16, _ACOL_AUD : _ACOL_AUD + 128],
                                        audio_t[:, b0 : b0 + bpc_c, a : a + wm],
                                        start=False,
                                        stop=True,
                                    )
                                    eng = nc.vector if ADD_ENG[c] == "dve" else nc.gpsimd
                                    eng.tensor_copy(
                                        out=x_next[:, :, o : o + wd], in_=pio[:, :, :wd]
                                    )
                                else:
                                    nc.tensor.matmul(
                                        pio[:, :, :wm],
                                        wall_t[:, col : col + 128],
                                        h[:, :, o : o + wm],
                                        start=True,
                                        stop=True,
                                    )
                                    a = lo - TB[i]
                                    eng = nc.vector if ADD_ENG[c] == "dve" else nc.gpsimd
                                    eng.tensor_add(
                                        out=x_next[:, :, o : o + wd],
                                        in0=x_t[:, :, a : a + wd],
                                        in1=pio[:, :, :wd],
                                    )
                        if i < NL - 1:
                            x_ts[c] = x_next

                # mixer: per chain, one 20-matmul PSUM group over the
                # persistent h slices (one bank, groups serialized by the
                # out-copy dependency), then copy to SBUF
                out_t = wpool.tile([16, BPC, 8], f32, name="out_t", tag="out_t")
                for c in range(NCHAINS):
                    b0 = c * bpc_c
                    pm = pmpool.tile([16, bpc_c, 8], f32, name=f"pm{c}", tag="pm")
                    for i in range(NL):
                        h, r = hs[c][i]
                        nc.tensor.matmul(
                            pm[:],
                            wall_t[:, _WCOL_MIX[i] : _WCOL_MIX[i] + 16],
                            h[:, :, h.shape[2] - 8 :],
                            start=(i == 0),
                            stop=(i == NL - 1),
                            skip_group_check=True,
                        )
                    nc.scalar.activation(
                        out=out_t[:, b0 : b0 + bpc_c, :],
                        in_=pm[:],
                        func=mybir.ActivationFunctionType.Copy,
                    )
                dst = bass.AP(
                    tensor=out_h,
                    offset=0,
                    ap=[[1, BLK], [128, BPC], [BLK, 8]],
                )
                nc.scalar.dma_start(out=dst, in_=out_t[:])

    _split_multi_waits(nc)
    return nc


def _get_nc():
    if "nc" not in _NC_CACHE:
        _NC_CACHE["nc"] = _build_nc()
    return _NC_CACHE["nc"]


# ------------------------------------------------------------------- public
def _block(sig, b0, ones_row=False):
    """[b, T] -> [16(+1), b, nblk] suffix-block layout starting at block b0.
    ones_row appends a constant-1 partition (bias carrier for the ctrl
    stationary's 17th row)."""
    nblk = NB - b0
    v = sig[:, b0 * BLK :].reshape(sig.shape[0], nblk, BLK)
    v = np.ascontiguousarray(v.transpose(2, 0, 1))
    if ones_row:
        v = np.concatenate([v, np.ones((1,) + v.shape[1:], v.dtype)], axis=0)
    return v.astype(NPDT)


def kernel(**inputs) -> np.ndarray:
    nc = _get_nc()
    host = _build_host_arrays(inputs)
    audio = np.asarray(inputs["audio"], np.float32)[:, :, 0]
    ctrl = np.asarray(inputs["ctrl"], np.float32)[:, :, 0]
    mix_b = float(np.asarray(inputs["mix_b"], np.float32)[0])

    in_maps = []
    for c in range(NCORES):
        sl = slice(c * BPC, (c + 1) * BPC)
        in_maps.append(
            {
                "audio": _block(audio[sl], TB[0]),
                "ctrl": _block(ctrl[sl], TB[1], ones_row=True),
                "wall": host["wall"],
                "aux": host["aux"],
            }
        )
    res = run_bass_kernel_spmd(nc, in_maps, core_ids=list(range(NCORES)))
    out = np.concatenate([res.results[c]["out"] for c in range(NCORES)], axis=0)
    return (out + mix_b).astype(np.float32)
